# revision 18
# baseline (speedup 1.0000x reference)
"""Trainium2 Bass kernel for GQA causal attention (nn_Attention).

Reference computation (B=2, S=2048, D=4096, H=32, KV=8, HD=128):
    q/k/v projections -> RoPE(q, k) -> GQA attention with additive mask
    -> softmax -> out projection.

Sharding: TP=4 over heads x DP=2 over batch on 8 NeuronCores.
Each core computes, for its batch b and head shard tp:
    Q^T = (x_b @ wq_tp)^T, K^T, V  (projections with RoPE folded via
    host-side even/odd weight-column reordering + on-device rotation)
    S^T = K^T . Q^T per head (scores, transposed layout)
    P^T = exp(S^T) * expmask_tile   (lazy softmax, no max subtraction)
    U^T = V^T-accumulated P^T, rowsums via ones-matmul
    att^T = U^T * (1/rowsum);  out_partial = att @ wo_tp
Host sums the 4 TP partials per batch (the row-parallel all-reduce).

All matmuls run in bf16 with fp32 PSUM accumulation (measured end-to-end
scale-relative absmax error ~5e-3 vs the fp32 reference).
"""

import os
import math
import numpy as np
import ml_dtypes

# ---------------------------------------------------------------- constants
B, S, D = 2, 2048, 4096
H, KV, HD = 32, 8, 128
N_REP = H // KV
TP, DP = 4, 2
N_CORES = TP * DP
HL = H // TP            # 8 local q heads
KVL = KV // TP          # 2 local kv heads
P = 128                 # partitions
KT = D // P             # 32 contraction tiles for projections
NJ_FULL = S // 512      # 4 seq chunks of 512
NST = S // P            # 16 seq tiles of 128
BF = ml_dtypes.bfloat16

# module-level handle for test harness introspection
last_results = None
_cache = {}


def _classify_mask(mask: np.ndarray):
    """Turn the additive mask into multiplicative per-tile factors.

    Returns (table, uniq) where table[i][j] is 'full' (factor==1
    everywhere), 'zero' (factor==0 everywhere -> tile skipped), or an
    index into uniq, the list of distinct [128,512] f32 factor tiles in
    S^T layout ([sk, sq]).
    """
    m = mask.astype(np.float64)
    rowmax = np.max(m, axis=1, keepdims=True)  # per-query max over keys
    rowmax = np.where(np.isfinite(rowmax), rowmax, 0.0)
    em = np.exp(m - rowmax)                    # [sq, sk] in [0, inf)
    emT = np.ascontiguousarray(em.T).astype(np.float32)  # [sk, sq]
    table = [[None] * NJ_FULL for _ in range(NST)]
    uniq = []
    keys = {}
    for j in range(NJ_FULL):
        first = True
        for i in range(NST):
            t = emT[i * P:(i + 1) * P, j * 512:(j + 1) * 512]
            if np.all(t == 1.0):
                table[i][j] = "full"
                first = False
                continue
            if np.all(t == 0.0):
                table[i][j] = "zero"
                continue
            cols1 = np.all(t == 1.0, axis=0)   # all-ones columns
            cols0 = np.all(t == 0.0, axis=0)   # all-zero columns
            # live range starts after leading all-zero cols (first tile of a
            # j-chunk must start at 0 so the PSUM bank is fully initialized)
            lo = 0
            if not first:
                while lo < 512 and cols0[lo]:
                    lo += 1
            hi = 512
            while hi > lo and cols1[hi - 1]:
                hi -= 1
            w = hi - lo
            sub = t[:, lo:hi]
            key = sub.tobytes()
            if key not in keys:
                keys[key] = len(uniq)
                pad = np.ones((P, 512), np.float32)
                pad[:, :w] = sub
                uniq.append(pad)
            table[i][j] = (lo, w, keys[key])
            first = False
    return table, uniq


def _rope_perm(n_heads):
    """Column permutation putting even rope dims first, odd second, per head."""
    perm = []
    for h in range(n_heads):
        perm += [h * HD + 2 * i for i in range(HD // 2)]
        perm += [h * HD + 2 * i + 1 for i in range(HD // 2)]
    return np.array(perm, dtype=np.int64)


def _build(table_sig, table, n_uniq):
    """Build + compile the SPMD Bass program for one mask classification."""
    import concourse.bass as bass
    import concourse.tile as tile
    import concourse.mybir as mybir
    from concourse import bacc

    bf = mybir.dt.bfloat16
    f32 = mybir.dt.float32
    Exp = mybir.ActivationFunctionType.Exp
    MULT = mybir.AluOpType.mult
    ADD = mybir.AluOpType.add

    nc = bacc.Bacc("TRN2", target_bir_lowering=False, debug=False,
                   enable_asserts=False, num_devices=N_CORES)

    xT_d = nc.dram_tensor("xT", [D, S], bf, kind="ExternalInput")
    wq_d = nc.dram_tensor("wq", [D, HL * HD], bf, kind="ExternalInput")
    wk_d = nc.dram_tensor("wk", [D, KVL * HD], bf, kind="ExternalInput")
    wv_d = nc.dram_tensor("wv", [D, KVL * HD], bf, kind="ExternalInput")
    wo_d = nc.dram_tensor("wo", [HL * HD, D], bf, kind="ExternalInput")
    cosf_d = nc.dram_tensor("cosf", [P, S], bf, kind="ExternalInput")
    ssf_d = nc.dram_tensor("ssf", [P, S], bf, kind="ExternalInput")
    ones_d = nc.dram_tensor("ones", [P, P], bf, kind="ExternalInput")
    em_d = [nc.dram_tensor(f"em{u}", [P, 512], bf, kind="ExternalInput")
            for u in range(n_uniq)]
    out_d = nc.dram_tensor("out", [S, D], f32, kind="ExternalOutput")

    with tile.TileContext(nc) as tc:
        with tc.tile_pool(name="consts", bufs=1) as cpool:
            cosf = cpool.tile([P, S], bf, tag="cosf", name="cosf")
            ssf = cpool.tile([P, S], bf, tag="ssf", name="ssf")
            ones = cpool.tile([P, P], bf, tag="ones", name="ones")
            nc.sync.dma_start(cosf[:], cosf_d[:, :])
            nc.sync.dma_start(ssf[:], ssf_d[:, :])
            nc.sync.dma_start(ones[:], ones_d[:, :])
            em_sb = []
            for u in range(n_uniq):
                t = cpool.tile([P, 512], bf, tag=f"em{u}", name=f"em{u}")
                nc.sync.dma_start(t[:], em_d[u][:, :])
                em_sb.append(t)

            if True:
                qkv_pool = tc.alloc_tile_pool(name="qkv", bufs=1)
                if True:
                    QT = [qkv_pool.tile([P, S], bf, tag=f"qt{h}", name=f"qt{h}") for h in range(HL)]
                    KTt = [qkv_pool.tile([P, S], bf, tag=f"kt{g}", name=f"kt{g}") for g in range(KVL)]
                    V = [qkv_pool.tile([P, KVL * HD], bf, tag=f"v{st}", name=f"v{st}") for st in range(NST)]

                    # ------------- phase A: projections + RoPE ------------
                    XH = 2 if NJ_FULL % 2 == 0 else 1
                    SH = S // XH
                    QG = 4 if HL % 4 == 0 else HL   # q heads per weight group
                    with tc.tile_pool(name="xt", bufs=1) as xt_pool, \
                         tc.tile_pool(name="wq", bufs=1) as wq_pool, \
                         tc.tile_pool(name="wk", bufs=1) as wk_pool, \
                         tc.tile_pool(name="wv", bufs=1) as wv_pool, \
                         tc.tile_pool(name="ropetmp", bufs=2) as rt_pool, \
                         tc.tile_pool(name="psA", bufs=3, space="PSUM") as psA, \
                         tc.tile_pool(name="psV", bufs=2, space="PSUM") as psV:
                        for half in range(XH):
                            s0 = half * SH
                            xt = []
                            for k in range(KT):
                                t = xt_pool.tile([P, SH], bf, tag=f"xt{k}", name=f"xt{k}")
                                nc.gpsimd.dma_start(t[:], xT_d[k * P:(k + 1) * P, s0:s0 + SH])
                                xt.append(t)

                            def rope_gen(dst, wt, coff, jj):
                                """dst[:, jj*512..] = rope((x @ w)[:, coff:coff+128])"""
                                ps = psA.tile([P, 512], f32, tag="psqk", name="psqk")
                                lo = jj * 512 - s0
                                for k in range(KT):
                                    nc.tensor.matmul(ps[:], wt[k][:, coff:coff + P],
                                                     xt[k][:, lo:lo + 512],
                                                     start=(k == 0), stop=(k == KT - 1))
                                qb = rt_pool.tile([P, 512], bf, tag="qb", name="qb")
                                nc.scalar.copy(qb[:], ps[:])
                                qsw = rt_pool.tile([P, 512], bf, tag="qsw", name="qsw")
                                nc.scalar.copy(qsw[0:64, :], ps[64:128, :])
                                nc.scalar.copy(qsw[64:128, :], ps[0:64, :])
                                t1 = rt_pool.tile([P, 512], bf, tag="t1", name="t1")
                                nc.vector.tensor_tensor(
                                    t1[:], qb[:], cosf[:, jj * 512:jj * 512 + 512], MULT)
                                t2 = rt_pool.tile([P, 512], bf, tag="t2", name="t2")
                                nc.vector.tensor_tensor(
                                    t2[:], qsw[:], ssf[:, jj * 512:jj * 512 + 512], MULT)
                                nc.vector.tensor_tensor(
                                    dst[:, jj * 512:jj * 512 + 512], t1[:], t2[:], ADD)

                            half_js = list(range(half * (NJ_FULL // XH),
                                                 (half + 1) * (NJ_FULL // XH)))
                            # V and K first so attention can start early
                            wvt = []
                            for k in range(KT):
                                t = wv_pool.tile([P, KVL * HD], bf, tag=f"wv{k}", name=f"wv{k}")
                                nc.sync.dma_start(t[:], wv_d[k * P:(k + 1) * P, :])
                                wvt.append(t)
                            for st in range(half * (NST // XH), (half + 1) * (NST // XH)):
                                ps = psV.tile([P, KVL * HD], f32, tag="psv", name="psv")
                                lo = st * P - s0
                                for k in range(KT):
                                    nc.tensor.matmul(ps[:], xt[k][:, lo:lo + P], wvt[k][:],
                                                     start=(k == 0), stop=(k == KT - 1))
                                nc.scalar.copy(V[st][:], ps[:])
                            wkt = []
                            for k in range(KT):
                                t = wk_pool.tile([P, KVL * HD], bf, tag=f"wk{k}", name=f"wk{k}")
                                nc.sync.dma_start(t[:], wk_d[k * P:(k + 1) * P, :])
                                wkt.append(t)
                            for g in range(KVL):
                                for jj in half_js:
                                    rope_gen(KTt[g], wkt, g * HD, jj)
                            for hg in range(HL // QG):
                                wqt = []
                                for k in range(KT):
                                    t = wq_pool.tile([P, QG * HD], bf, tag=f"wq{k}", name=f"wq{k}")
                                    nc.sync.dma_start(
                                        t[:], wq_d[k * P:(k + 1) * P,
                                                   hg * QG * HD:(hg + 1) * QG * HD])
                                    wqt.append(t)
                                for h in range(hg * QG, (hg + 1) * QG):
                                    for jj in half_js:
                                        rope_gen(QT[h], wqt, (h - hg * QG) * HD, jj)

                    # ------------- phase B: attention ---------------------
                    ut_pool = tc.alloc_tile_pool(name="ut", bufs=1)
                    UT = [ut_pool.tile([P, S], bf, tag=f"ut{h}", name=f"ut{h}")
                          for h in range(HL)]
                    wo_pool = tc.alloc_tile_pool(name="wo", bufs=1)
                    ob_pool = tc.alloc_tile_pool(name="ob", bufs=2)
                    psC = tc.alloc_tile_pool(name="psC", bufs=2, space="PSUM")
                    wot = []
                    for h in range(HL):
                        t = wo_pool.tile([P, D], bf, tag=f"wo{h}", name=f"wo{h}")
                        nc.sync.dma_start(t[:], wo_d[h * P:(h + 1) * P, :])
                        wot.append(t)
                    with tc.tile_pool(name="pt", bufs=3) as pt_pool, \
                         tc.tile_pool(name="rnorm", bufs=2) as rn_pool, \
                         tc.tile_pool(name="psS", bufs=3, space="PSUM") as psS_pool, \
                         tc.tile_pool(name="psU", bufs=2, space="PSUM") as psU_pool, \
                         tc.tile_pool(name="psR", bufs=1, space="PSUM") as psR_pool:
                        for j in range(NJ_FULL):
                            jsl = slice(j * 512, j * 512 + 512)
                            inc = [i for i in range(NST) if table[i][j] != "zero"]
                            for h in range(HL):
                                g = h // N_REP
                                if not inc:
                                    nc.vector.memset(UT[h][:, jsl], 0.0)
                                    continue
                                psU = psU_pool.tile([P, 512], f32, tag="psu", name="psu")
                                psR = psR_pool.tile([32, 512], f32, tag="psr", name="psr")
                                for idx, i in enumerate(inc):
                                    cls = table[i][j]
                                    lo = 0 if cls == "full" else cls[0]
                                    n = 512 - lo
                                    psS = psS_pool.tile([P, 512], f32, tag="pss", name="pss")
                                    nc.tensor.matmul(psS[:, lo:],
                                                     KTt[g][:, i * P:(i + 1) * P],
                                                     QT[h][:, j * 512 + lo:j * 512 + 512],
                                                     start=True, stop=True)
                                    pt = pt_pool.tile([P, 512], bf, tag="pt", name="pt")
                                    nc.scalar.activation(pt[:, lo:], psS[:, lo:], Exp)
                                    if cls != "full":
                                        _, w, u = cls
                                        nc.vector.tensor_tensor(
                                            pt[:, lo:lo + w], pt[:, lo:lo + w],
                                            em_sb[u][:, 0:w], MULT)
                                    fl = (idx == 0)
                                    ll = (idx == len(inc) - 1)
                                    nc.tensor.matmul(psU[:, lo:], V[i][:, g * HD:(g + 1) * HD],
                                                     pt[:, lo:], start=fl, stop=ll)
                                    nc.tensor.matmul(psR[:, lo:], ones[:, 0:32], pt[:, lo:],
                                                     start=fl, stop=ll)
                                # normalize inline: UT = psU * (1/rowsum)
                                r32 = rn_pool.tile([32, 512], f32, tag="r32", name="r32")
                                nc.vector.reciprocal_approx_fast(r32[:], psR[:])
                                rb = rn_pool.tile([P, 512], f32, tag="rb", name="rb")
                                nc.gpsimd.dma_start(rb[0:32, :], r32[:])
                                nc.gpsimd.dma_start(rb[32:64, :], r32[:])
                                nc.gpsimd.dma_start(rb[64:128, :], rb[0:64, :])
                                nc.vector.tensor_tensor(UT[h][:, jsl], psU[:], rb[:], MULT)

                # ------------- phase C: out projection --------------------
                OBH = 2 if D >= 2048 else 1
                OBW = D // OBH
                NDC = OBW // 512
                for st in range(NST):
                    for half in range(OBH):
                        ob = ob_pool.tile([P, OBW], f32, tag="ob", name="ob")
                        for dch in range(NDC):
                            dc = half * NDC + dch
                            psO = psC.tile([P, 512], f32, tag="pso", name="pso")
                            for h in range(HL):
                                nc.tensor.matmul(
                                    psO[:], UT[h][:, st * P:(st + 1) * P],
                                    wot[h][:, dc * 512:dc * 512 + 512],
                                    start=(h == 0), stop=(h == HL - 1))
                            dsl = slice(dch * 512, dch * 512 + 512)
                            nc.vector.tensor_copy(ob[:, dsl], psO[:])
                        nc.gpsimd.dma_start(
                            out_d[st * P:(st + 1) * P,
                                  half * OBW:(half + 1) * OBW], ob[:])
                psC.release()
                ob_pool.release()
                wo_pool.release()
                ut_pool.release()

                qkv_pool.release()

    nc.compile()
    return nc


def kernel(x, freqs_cos, freqs_sin, mask, wq, wk, wv, wo):
    global last_results
    from concourse.bass_utils import run_bass_kernel_spmd

    x = np.asarray(x)
    mask = np.asarray(mask, dtype=np.float32)
    table, uniq = _classify_mask(mask)
    sig = tuple(tuple(r) for r in table), len(uniq)
    key = ("k", sig)
    if key not in _cache:
        _cache[key] = _build(sig, table, len(uniq))
    nc = _cache[key]

    qperm = _rope_perm(H)
    kperm = _rope_perm(KV)
    wq_r = np.asarray(wq)[:, qperm]
    wk_r = (np.asarray(wk) * (1.0 / math.sqrt(HD)))[:, kperm]
    wv_n = np.asarray(wv)
    wo_n = np.asarray(wo)

    cosT = np.asarray(freqs_cos).T.astype(np.float32)     # [64, S]
    sinT = np.asarray(freqs_sin).T.astype(np.float32)
    cosf = np.concatenate([cosT, cosT], axis=0).astype(BF)  # [128, S]
    ssf = np.concatenate([-sinT, sinT], axis=0).astype(BF)
    ones = np.ones((P, P), dtype=BF)

    in_maps = []
    for c in range(N_CORES):
        b, tp = c // TP, c % TP
        m = {
            "xT": np.ascontiguousarray(x[b].T).astype(BF),
            "wq": np.ascontiguousarray(wq_r[:, tp * HL * HD:(tp + 1) * HL * HD]).astype(BF),
            "wk": np.ascontiguousarray(wk_r[:, tp * KVL * HD:(tp + 1) * KVL * HD]).astype(BF),
            "wv": np.ascontiguousarray(wv_n[:, tp * KVL * HD:(tp + 1) * KVL * HD]).astype(BF),
            "wo": np.ascontiguousarray(wo_n[tp * HL * HD:(tp + 1) * HL * HD, :]).astype(BF),
            "cosf": cosf, "ssf": ssf, "ones": ones,
        }
        for u, t in enumerate(uniq):
            m[f"em{u}"] = t.astype(BF)
        in_maps.append(m)

    trace = bool(os.environ.get("BASS_TRACE"))
    last_results = run_bass_kernel_spmd(
        nc, in_maps, core_ids=list(range(N_CORES)), trace=trace)

    out = np.zeros((B, S, D), dtype=np.float32)
    for c in range(N_CORES):
        out[c // TP] += last_results.results[c]["out"]
    return out
